# revision 1
# baseline (speedup 1.0000x reference)
"""MultiHuberLoss Trainium2 kernel (sorted extraction + PE B-term).

Reference (per element, with m = +x at the target class, -x elsewhere):
    hinge = max(0, 1 - m);  loss = where(m >= -1, hinge^2, -4m);  out = sum(loss)/N

Math (exact identities):
  Main pass treats every element as non-target (m = -x):
      F(-x) = (clamp(x,-1,1) + 1)^2 + 4*max(x, 1) - 4
  Per-row correction for the target column t: F(x_t) - F(-x_t) = -4 * x_t
  So per core:
      S = sum_ij (v+1)^2  +  4*sum_ij u  -  4*ROWS*C  -  4*sum_i x[i, t_i]
      with v = clamp(x,-1,1), u = max(x,1)

Engine split (per 4MB tile, all hidden under the ~82us DMA stream):
  - DVE:   v = clamp(x,-1,1) -> bf16 (tensor_scalar 2x mode)
           u = max(x,1)      -> bf16 (tensor_scalar 2x mode)
  - ACT:   Square(v + 1) with fused accum -> per-partition sums accA
  - PE:    column sums of u via ones^T @ u_chunk accumulated into one
           PSUM bank across all chunks/tiles (B term)
  - GPSIMD: target extraction.  Host pre-sorts rows by target class so
    that each 16-partition-group x row-position slot holds 16 rows with
    the SAME target for j in [0,48) -> ONE ap_gather per 8-row tile
    extracts them (shared index per group).  j in [48,64) holds the
    mixed leftovers -> 16 per-partition indirect-DMA columns.

The row permutation is applied on host (the loss is a plain sum over
rows, so it is permutation invariant); bf16 intermediates are safely
inside the 2e-2 tolerance (measured rel err ~1e-5).
"""

import numpy as np

import concourse.bacc as bacc
import concourse.bass as bass
import concourse.mybir as mybir
from concourse.bass_utils import run_bass_kernel_spmd
from concourse.tile import TileContext

N_TOTAL = 65536
C = 1000
N_CORES = 8
ROWS = N_TOTAL // N_CORES  # 8192 rows per core
P = 128                    # partitions
JPP = ROWS // P            # 64 row-positions per partition
FREE = JPP * C             # 64000 f32 per partition
NGRP = P // 16             # 8 gpsimd 16-partition groups
J_UNIF = 56                # row-positions extracted via ap_gather
J_MIX = JPP - J_UNIF       # 8 row-positions via indirect DMA
RPT = 4                    # row-positions per big tile (FD=4000)
NT_BIG = J_UNIF // RPT     # 12 ap_gather tiles
# tile free-dim sizes: 2MB tiles (deep 5-buffer pipeline absorbs the
# per-tile completion jitter of the slow DMA engines 7/15) plus a
# tapered tail to shorten the critical chain after the last DMA lands
TILE_FDS = [4000] * 15 + [2000] * 2
assert sum(TILE_FDS) == FREE

f32 = mybir.dt.float32
bf16 = mybir.dt.bfloat16
i32 = mybir.dt.int32
i16 = mybir.dt.int16
Alu = mybir.AluOpType
AF = mybir.ActivationFunctionType


def build_program():
    nc = bacc.Bacc(
        "TRN2", target_bir_lowering=False, debug=False, num_devices=N_CORES
    )
    x = nc.dram_tensor("x", [ROWS, C], f32, kind="ExternalInput")
    # ap_gather shared indices: [128, NT_BIG] int16
    idx = nc.dram_tensor("idx", [P, NT_BIG], i16, kind="ExternalInput")
    # per-row flat offsets for the mixed region: [128, J_MIX] int32
    og = nc.dram_tensor("og", [P, J_MIX], i32, kind="ExternalInput")
    out = nc.dram_tensor("out", [1, 1], f32, kind="ExternalOutput")

    x_flat = x.ap().rearrange("(p j) c -> p (j c)", p=P)       # [128, 64000]
    x_lin = x.ap().rearrange("a (b one) -> (a b) one", one=1)  # [8192000, 1]

    NT = len(TILE_FDS)

    with TileContext(nc) as tc:
        with (
            tc.tile_pool(name="xp", bufs=5) as xp,
            tc.tile_pool(name="vp", bufs=2) as vp,
            tc.tile_pool(name="up", bufs=2) as up,
            tc.tile_pool(name="scr", bufs=1) as scr,
            tc.tile_pool(name="small", bufs=1) as small,
            tc.tile_pool(name="psp", bufs=1, space="PSUM") as psp,
        ):
            # ACT-only discard output for the Square pass (one buffer,
            # written only by Scalar so no cross-engine WAW stalls)
            sq_scr = scr.tile([P, max(TILE_FDS)], bf16, tag="sq_scr")
            # x tile 0 queued first so the big stream leads the Sync queue
            tiles = []
            xt0 = xp.tile([P, TILE_FDS[0]], f32)
            nc.sync.dma_start(out=xt0[:], in_=x_flat[:, 0:TILE_FDS[0]])
            tiles.append(xt0)

            idxs = small.tile([P, NT_BIG], i16, tag="idxs")
            nc.sync.dma_start(out=idxs[:], in_=idx.ap())
            offs = small.tile([P, J_MIX], i32, tag="offs")
            nc.sync.dma_start(out=offs[:], in_=og.ap())
            ones_bf = small.tile([P, 1], bf16, tag="ones_bf")
            nc.vector.memset(ones_bf[:], 1.0)
            ones_f = small.tile([P, 1], f32, tag="ones_f")
            nc.vector.memset(ones_f[:], 1.0)

            accA = small.tile([P, NT], f32, tag="accA")
            # per big tile: cols [16t,16t+8) = ap_gather targets,
            # [16t+8, 16t+8+k_t) = indirect-DMA columns. The indirect
            # writes OVERWRITE ap_gather pad slots on purpose: the WAW dep
            # pins each indirect after its tile's ap_gather, which stops
            # the tile scheduler from hoisting the whole indirect chain to
            # the front of the GPSIMD queue (head-of-line blocking the
            # ap_gathers and stalling buffer recycling -- measured 30us).
            gat = small.tile([P, NT_BIG * 16], f32, tag="gat")
            psB = psp.tile([1, 512], f32, tag="psB")

            def emit_indirect(j, t, m):
                # mixed-region gather column j -> pad slot m of tile t
                nc.gpsimd.indirect_dma_start(
                    out=gat[:, t * 16 + RPT + m:t * 16 + RPT + m + 1],
                    out_offset=None,
                    in_=x_lin,
                    in_offset=bass.IndirectOffsetOnAxis(
                        ap=offs[:, j:j + 1], axis=0
                    ),
                )

            # ---- main streaming loop ----
            off = 0
            n_chunks_total = sum(fd // 500 for fd in TILE_FDS)
            ci = 0
            for t, fd in enumerate(TILE_FDS):
                if t == 0:
                    xt = tiles[0]
                else:
                    xt = xp.tile([P, fd], f32)
                    nc.sync.dma_start(
                        out=xt[:], in_=x_flat[:, off:off + fd]
                    )
                v = vp.tile([P, fd], bf16)
                nc.vector.tensor_scalar(
                    v[:], xt[:], -1.0, 1.0, Alu.max, Alu.min
                )
                nc.scalar.activation(
                    sq_scr[:, 0:fd],
                    v[:],
                    AF.Square,
                    bias=1.0,
                    scale=1.0,
                    accum_out=accA[:, t:t + 1],
                )
                last = t == NT - 1
                u = up.tile([P, fd], f32 if last else bf16)
                if last:
                    u_lastf = u
                nc.vector.tensor_scalar(u[:], xt[:], 1.0, None, Alu.max)
                for c in range(fd // 500):
                    nc.tensor.matmul(
                        out=psB[:, 0:500],
                        lhsT=ones_f[:] if last else ones_bf[:],
                        rhs=u[:, c * 500:(c + 1) * 500],
                        start=(ci == 0),
                        stop=(ci == n_chunks_total - 1),
                    )
                    ci += 1
                if t < NT_BIG:
                    # targets of the RPT uniform row-positions of this tile.
                    # GPSIMD is strict-FIFO: the ap_gather for tile t must
                    # sit early in the queue (it holds tile t's buffer), so
                    # the indirect columns are interleaved 1-2 per tile to
                    # keep per-tile GPSIMD work under the DMA period.
                    nc.gpsimd.ap_gather(
                        out_ap=gat[:, t * 16:(t + 1) * 16],
                        in_ap=xt[:],
                        idxs_ap=idxs[:, t:t + 1],
                        channels=P, num_elems=fd, d=1, num_idxs=16,
                    )
                    lo = (t * J_MIX) // NT_BIG
                    hi = ((t + 1) * J_MIX) // NT_BIG
                    for j in range(lo, hi):
                        emit_indirect(j, t, j - lo)
                    # zero the remaining pad slots (in-order on GPSIMD, so
                    # no cross-engine stall) -> ONE whole-tile reduce below
                    nc.gpsimd.memset(
                        gat[:, t * 16 + RPT + (hi - lo):(t + 1) * 16], 0.0
                    )
                off += fd
            assert ci == n_chunks_total

            # ---- final combine ----
            # Batched reduces pinned to the END of the schedule via a read
            # of sq_scr (last written by the final Square): a plain reduce
            # placed mid-DVE-queue would wait on its gather inputs and
            # stall clamp/u of later tiles (measured 14-30us of DMA
            # stalls / DVE lag). op0=bypass passes in0 through; the in1
            # read exists only for its scheduling dependency.
            rG = u_lastf[:, 0:1]
            nc.vector.reduce_sum(rG, gat[:], axis=mybir.AxisListType.X)
            rA = u_lastf[:, 1:2]
            nc.vector.reduce_sum(rA, accA[:], axis=mybir.AxisListType.X)
            # u1 = rA - 4*rG  (per-partition)
            u1 = small.tile([P, 1], f32, tag="u1")
            nc.vector.scalar_tensor_tensor(
                out=u1[:], in0=rG, scalar=-4.0, in1=rA,
                op0=Alu.mult, op1=Alu.add,
            )
            psS = psp.tile([1, 8], f32, tag="psS")
            nc.tensor.matmul(
                out=psS[:, 0:1], lhsT=ones_f[:], rhs=u1[:],
                start=True, stop=True,
            )
            # sB = sum over the accumulated B bank
            sb_scr = small.tile([1, 500], f32, tag="sb_scr")
            sB = small.tile([1, 1], f32, tag="sB")
            nc.scalar.activation(
                sb_scr[:], psB[:, 0:500], AF.Identity,
                bias=0.0, scale=1.0, accum_out=sB[:],
            )
            # tmp = 4*sB + psS ;  res = tmp/N - 4*ROWS*C/N
            tmp = small.tile([1, 1], f32, tag="tmp")
            nc.vector.scalar_tensor_tensor(
                out=tmp[:], in0=sB[:], scalar=4.0, in1=psS[:, 0:1],
                op0=Alu.mult, op1=Alu.add,
            )
            biasc = -4.0 * ROWS * C / N_TOTAL  # = -500.0
            bias_t = small.tile([1, 1], f32, tag="bias")
            nc.vector.memset(bias_t[:], biasc)
            res = small.tile([1, 1], f32, tag="res")
            nc.scalar.activation(
                res[:], tmp[:], AF.Identity,
                bias=bias_t[:], scale=1.0 / N_TOTAL,
            )
            nc.sync.dma_start(out=out.ap(), in_=res[:])

    nc.compile()
    return nc


# ---------------- host-side placement ----------------

def build_placement(target):
    """Sort rows by target class and pack them so every 16-partition-group
    slot with j<J_UNIF is target-uniform. Returns (perm, idx16, og) where
    perm maps dest global row -> src row."""
    target = np.asarray(target).astype(np.int64)
    order = np.argsort(target, kind="stable")
    tsort = target[order]
    changes = np.flatnonzero(np.diff(tsort)) + 1
    starts = np.concatenate(([0], changes))
    ends = np.concatenate((changes, [N_TOTAL]))

    unif_list = []
    leftover_parts = []
    for s, e in zip(starts, ends):
        nfull = (e - s) // 16
        if nfull:
            unif_list.append(order[s:s + 16 * nfull].reshape(nfull, 16))
        if s + 16 * nfull < e:
            leftover_parts.append(order[s + 16 * nfull:e])
    unif = (
        np.concatenate(unif_list, axis=0)
        if unif_list else np.empty((0, 16), np.int64)
    )
    leftover = (
        np.concatenate(leftover_parts)
        if leftover_parts else np.empty(0, np.int64)
    )
    assert leftover.size % 16 == 0
    mixed = leftover.reshape(-1, 16)

    n_unif_needed = N_CORES * NGRP * J_UNIF  # 3072
    assert unif.shape[0] >= n_unif_needed, (
        f"not enough uniform 16-row groups: {unif.shape[0]} < {n_unif_needed}"
    )
    spill = unif[n_unif_needed:]
    unif = unif[:n_unif_needed]
    mix = np.concatenate([mixed, spill], axis=0)
    assert mix.shape[0] == N_CORES * NGRP * J_MIX  # 1024 exactly

    perm = np.empty(N_TOTAL, np.int64)
    q = np.arange(16)
    # uniform slots: chunk index ci -> (core, g, j) with j fastest
    ci = np.arange(n_unif_needed)
    core = ci // (NGRP * J_UNIF)
    rem = ci % (NGRP * J_UNIF)
    g = rem // J_UNIF
    j = rem % J_UNIF
    dest = (core * ROWS)[:, None] + (16 * g[:, None] + q[None, :]) * JPP \
        + j[:, None]
    perm[dest.ravel()] = unif.ravel()
    # mixed slots
    mi = np.arange(mix.shape[0])
    core = mi // (NGRP * J_MIX)
    rem = mi % (NGRP * J_MIX)
    g = rem // J_MIX
    j = J_UNIF + rem % J_MIX
    dest = (core * ROWS)[:, None] + (16 * g[:, None] + q[None, :]) * JPP \
        + j[:, None]
    perm[dest.ravel()] = mix.ravel()

    tgt_perm = target[perm].reshape(N_CORES, P, JPP)
    # idx16[core, 16g+m, t] = m*C + tgt(g, RPT*t+m) for m < RPT; pad 0
    idx16 = np.zeros((N_CORES, P, NT_BIG), np.int16)
    m = np.arange(RPT)
    t_ar = np.arange(NT_BIG)
    for gg in range(NGRP):
        # tgt at (core, partition 16*gg, j=RPT*t+m): [N_CORES, NT_BIG, RPT]
        tg = tgt_perm[:, 16 * gg, :J_UNIF].reshape(N_CORES, NT_BIG, RPT)
        vals = (m[None, None, :] * C + tg).astype(np.int16)
        idx16[:, 16 * gg + m, :] = vals.transpose(0, 2, 1)
    # og[core, p, col] = (p*JPP + J_UNIF+col)*C + tgt
    p_ar = np.arange(P)
    col = np.arange(J_MIX)
    og = (
        (p_ar[None, :, None] * JPP + J_UNIF + col[None, None, :]) * C
        + tgt_perm[:, :, J_UNIF:]
    ).astype(np.int32)
    return perm, idx16, og


_NC_CACHE = None
LAST_RESULTS = None


def kernel(input, target):
    global _NC_CACHE, LAST_RESULTS
    x = np.ascontiguousarray(np.asarray(input, dtype=np.float32))
    tg = np.ascontiguousarray(np.asarray(target).astype(np.int64))
    assert x.shape == (N_TOTAL, C), x.shape
    assert tg.shape == (N_TOTAL,), tg.shape

    if _NC_CACHE is None:
        _NC_CACHE = build_program()
    nc = _NC_CACHE

    perm, idx16, og = build_placement(tg)
    x_perm = np.ascontiguousarray(x[perm])

    in_maps = [
        {
            "x": x_perm[c * ROWS:(c + 1) * ROWS],
            "idx": idx16[c],
            "og": og[c],
        }
        for c in range(N_CORES)
    ]
    res = run_bass_kernel_spmd(nc, in_maps, core_ids=list(range(N_CORES)))
    LAST_RESULTS = res
    total = np.float32(0.0)
    for r in res.results:
        total += np.float32(r["out"].reshape(()))
    return np.asarray(total, dtype=np.float32)


if __name__ == "__main__":
    rng = np.random.default_rng(0)
    xs = rng.standard_normal((N_TOTAL, C), dtype=np.float32)
    ts = rng.integers(0, C, size=(N_TOTAL,)).astype(np.int64)
    got = kernel(xs, ts)
    m = np.where(np.arange(C)[None, :] == ts[:, None], xs, -xs)
    hinge = np.maximum(0.0, 1.0 - m)
    loss = np.where(m >= -1.0, hinge * hinge, -4.0 * m)
    want = loss.sum(dtype=np.float64) / N_TOTAL
    print("got", got, "want", want, "rel", abs(got - want) / abs(want))

